# revision 16
# baseline (speedup 1.0000x reference)
"""
Trainium2 Bass kernel for nn_MetaAttention.

Computation (per batch b):
    rowsum[h,i]     = sum_j m[b,h,i,j]
    aggregated[i,j] = sum_h rowsum[h,i] * m[b,h,i,j]
    out[b]          = softmax(aggregated.flatten()).reshape(N, N)

Sharding: pure data parallel over B=16 across 8 cores (2 batches/core).

Per-core strategy (memory regime; measured all-8-core DMA rates):
  - Row layout: 128 partitions x 6 contiguous rows (rows 0..767; row =
    6p + s) plus a 16-partition remainder (rows 768..783). Loads are one
    DMA per HEAD PAIR [128, 2, 6, 784] -> two 18.8 KB contiguous
    descriptors per partition, which benches at ~373 GB/s/core vs
    ~215 GB/s for the 112-partition per-row-tile layout.
  - Scale-accumulate over heads split by SLOT: slots 0-2 accumulate on
    PE (diag(rowsum) matmul into persistent PSUM accs, fp32), slots 3-5
    on SBUF engines (DVE scalar_tensor_tensor alternating with ACT
    scale-copy + GPSIMD add), remainder rows on DVE.
  - Online softmax: each slot gets exp(x - slot_max) + fused sum as soon
    as its head loop finishes (PE slots exp directly out of PSUM); the
    global correction exp(slot_max - M) / Z folds into the final scale,
    so only a short scalar chain remains after the last load.
  - Output stored as bf16 (host upcasts; rel tol is 2e-2) halving store
    traffic; stores + small remainder loads ride the scalar HWDGE queue,
    bulk loads the sync queue.
  - Last batch's final head streams in 3 slot-pair pieces so the
    post-last-load tail is one small finalize + chain + scaled stores.
"""

import numpy as np

B, H, N = 16, 12, 784
NCORES = 8
BPC = B // NCORES          # batches per core
P = 128                    # main partitions
SLOTS = 6                  # full row-slots: row = 6p + s (rows 0..767)
RSLOT = SLOTS              # remainder slot index in mh tiles
REMP = 16                  # remainder rows 768..783 live on partitions 0..15
PE_SLOTS = (0, 1, 2)       # slots accumulated on PE/PSUM
SB_SLOTS = (3, 4, 5)       # slots accumulated on SBUF engines
JSPLITS = [(0, 512), (512, 272)]   # matmul free-dim splits (PSUM bank aligned)
RS_ACT_PAIRS = (2, 4)      # head-pairs whose rowsum runs on ACT (rest DVE)

LAST_RESULT = None  # BassKernelResults of the most recent kernel() call


def build_program():
    import concourse.bacc as bacc
    import concourse.tile as tile
    from concourse import mybir

    f32 = mybir.dt.float32
    bf16 = mybir.dt.bfloat16
    AX = mybir.AxisListType.X
    OP = mybir.AluOpType
    AF = mybir.ActivationFunctionType
    nc = bacc.Bacc("TRN2")

    x = nc.dram_tensor("x", [BPC, H, N, N], f32, kind="ExternalInput")
    ident = nc.dram_tensor("ident", [P, P], f32, kind="ExternalInput")
    y = nc.dram_tensor("y", [BPC, N, N], bf16, kind="ExternalOutput")

    with tile.TileContext(nc) as tc:
        with (
            tc.tile_pool(name="mh", bufs=3) as mh_pool,
            tc.tile_pool(name="agg", bufs=2) as agg_pool,
            tc.tile_pool(name="acc", bufs=4, space="PSUM") as acc_pool,
            tc.tile_pool(name="dg", bufs=4) as dg_pool,
            tc.tile_pool(name="scr", bufs=2) as scr_pool,
            tc.tile_pool(name="outp", bufs=2) as out_pool,
            tc.tile_pool(name="small", bufs=4) as small_pool,
            tc.tile_pool(name="consts", bufs=1) as const_pool,
        ):
            ident_sb = const_pool.tile([P, P], f32)
            nc.sync.dma_start(out=ident_sb, in_=ident[:, :])
            ones_sb = const_pool.tile([P, P], f32)
            nc.vector.memset(ones_sb, 1.0)

            def unit(h, s, mh_s, rs_s, agg, accs, tail=False):
                """Scale-accumulate head h's slot s: agg_s += rs*m."""
                if s in PE_SLOTS:
                    dg = dg_pool.tile([P, P], f32, tag="dg")
                    nc.scalar.activation(out=dg, in_=ident_sb, func=AF.Copy,
                                         bias=0.0, scale=rs_s)
                    acc = accs[PE_SLOTS.index(s)]
                    for j0, jn in JSPLITS:
                        nc.tensor.matmul(acc[:, j0:j0 + jn], lhsT=dg,
                                         rhs=mh_s[:, j0:j0 + jn],
                                         start=(h == 0), stop=(h == H - 1))
                    return
                a = agg[:, s, :]
                if h == 0:
                    nc.vector.tensor_scalar_mul(out=a, in0=mh_s, scalar1=rs_s)
                elif tail:
                    # keep the slow GPSIMD hop off the critical tail
                    if s == SB_SLOTS[-1]:
                        sc2 = scr_pool.tile([P, N], f32, tag="scr")
                        nc.scalar.activation(out=sc2, in_=mh_s, func=AF.Copy,
                                             bias=0.0, scale=rs_s)
                        nc.vector.tensor_tensor(out=a, in0=sc2, in1=a,
                                                op=OP.add)
                    else:
                        nc.vector.scalar_tensor_tensor(
                            out=a, in0=mh_s, scalar=rs_s, in1=a,
                            op0=OP.mult, op1=OP.add)
                elif (h + s) % 2 == 0:
                    nc.vector.scalar_tensor_tensor(
                        out=a, in0=mh_s, scalar=rs_s, in1=a,
                        op0=OP.mult, op1=OP.add)
                else:
                    sc2 = scr_pool.tile([P, N], f32, tag="scr")
                    nc.scalar.activation(out=sc2, in_=mh_s, func=AF.Copy,
                                         bias=0.0, scale=rs_s)
                    nc.gpsimd.tensor_tensor(out=a, in0=sc2, in1=a, op=OP.add)

            def rem_unit(h, mh, rs, aggr, late=False):
                """Remainder rows (768+p, p<16) for head h."""
                m_ = mh[0:REMP, RSLOT, :] if len(mh.shape) == 3 \
                    else mh[0:REMP, 0, RSLOT, :]
                if h == 0:
                    nc.vector.tensor_scalar_mul(out=aggr, in0=m_, scalar1=rs)
                elif late and h % 2 == 1 and h < H - 1:
                    sc2 = scr_pool.tile([P, N], f32, tag="scr")
                    nc.scalar.activation(out=sc2[0:REMP, :], in_=m_,
                                         func=AF.Copy, bias=0.0, scale=rs)
                    nc.gpsimd.tensor_tensor(out=aggr, in0=sc2[0:REMP, :],
                                            in1=aggr, op=OP.add)
                else:
                    nc.vector.scalar_tensor_tensor(
                        out=aggr, in0=m_, scalar=rs, in1=aggr,
                        op0=OP.mult, op1=OP.add)

            def finalize_slot(s, agg, accs, nm, sums):
                """Online softmax for one finished slot: negated max + exp."""
                src = accs[PE_SLOTS.index(s)][:, 0:N] if s in PE_SLOTS \
                    else agg[:, s, :]
                nc.vector.tensor_reduce(out=nm[:, s:s + 1], in_=src, axis=AX,
                                        op=OP.max, negate=True)
                nc.scalar.activation(out=agg[:, s, :], in_=src, func=AF.Exp,
                                     bias=nm[:, s:s + 1], scale=1.0,
                                     accum_out=sums[:, s:s + 1])

            for b in range(BPC):
                tailb = b == BPC - 1
                agg = agg_pool.tile([P, SLOTS, N], f32, tag="agg")
                aggr = small_pool.tile([REMP, N], f32, tag="aggr", bufs=2)
                nm = small_pool.tile([P, SLOTS], f32, tag="nm", bufs=2)
                nmr = small_pool.tile([REMP, 1], f32, tag="nmr", bufs=2)
                sums = small_pool.tile([P, SLOTS], f32, tag="sums", bufs=2)
                sumr = small_pool.tile([REMP, 1], f32, tag="sumr", bufs=2)
                accs = [acc_pool.tile([P, 1024], f32, tag="acc",
                                      name=f"acc_{b}_{s}") for s in PE_SLOTS]

                npairs = H // 2 - 1 if tailb else H // 2
                for hp in range(npairs):
                    mh = mh_pool.tile([P, 2, SLOTS + 1, N], f32, tag="mh")
                    src = x[b, 2 * hp:2 * hp + 2, 0:P * SLOTS, :].rearrange(
                        "h (p r) j -> p h r j", p=P)
                    nc.sync.dma_start(out=mh[:, :, 0:SLOTS, :], in_=src)
                    srcr = x[b, 2 * hp:2 * hp + 2, P * SLOTS:N, :].rearrange(
                        "h p j -> p h j")
                    nc.scalar.dma_start(out=mh[0:REMP, :, RSLOT, :], in_=srcr)

                    rs = small_pool.tile([P, 2, SLOTS + 1], f32, tag="rs",
                                         bufs=4)
                    if hp in RS_ACT_PAIRS or (tailb and hp == 1):
                        scr = scr_pool.tile([P, N], f32, tag="scr")
                        for m in range(2):
                            for s in range(SLOTS):
                                nc.scalar.activation(
                                    out=scr, in_=mh[:, m, s, :], func=AF.Copy,
                                    bias=0.0, scale=1.0,
                                    accum_out=rs[:, m, s:s + 1])
                        nc.vector.tensor_reduce(
                            out=rs[0:REMP, :, RSLOT:RSLOT + 1],
                            in_=mh[0:REMP, :, RSLOT, :], axis=AX, op=OP.add)
                    else:
                        for m in range(2):
                            for s in range(SLOTS):
                                nc.vector.tensor_reduce(
                                    out=rs[:, m, s:s + 1], in_=mh[:, m, s, :],
                                    axis=AX, op=OP.add)
                        nc.vector.tensor_reduce(
                            out=rs[0:REMP, :, RSLOT:RSLOT + 1],
                            in_=mh[0:REMP, :, RSLOT, :], axis=AX, op=OP.add)
                    for m in range(2):
                        h = 2 * hp + m
                        for s in range(SLOTS):
                            unit(h, s, mh[:, m, s, :], rs[:, m, s:s + 1],
                                 agg, accs)
                        rem_unit(h, mh[:, m], rs[0:REMP, m, RSLOT:RSLOT + 1],
                                 aggr, late=tailb)
                        if h == H - 1:
                            for s in range(SLOTS):
                                finalize_slot(s, agg, accs, nm, sums)

                if tailb:
                    # head H-2: single-slab load
                    mh = mh_pool.tile([P, 1, SLOTS + 1, N], f32, tag="mh")
                    src = x[b, H - 2, 0:P * SLOTS, :].rearrange(
                        "(p r) j -> p r j", p=P)
                    nc.sync.dma_start(out=mh[:, 0, 0:SLOTS, :], in_=src)
                    srcr = x[b, H - 2, P * SLOTS:N, :]
                    nc.scalar.dma_start(out=mh[0:REMP, 0, RSLOT, :], in_=srcr)
                    rs = small_pool.tile([P, 1, SLOTS + 1], f32, tag="rs",
                                         bufs=4)
                    for s in range(SLOTS):
                        nc.vector.tensor_reduce(out=rs[:, 0, s:s + 1],
                                                in_=mh[:, 0, s, :],
                                                axis=AX, op=OP.add)
                    nc.vector.tensor_reduce(
                        out=rs[0:REMP, :, RSLOT:RSLOT + 1],
                        in_=mh[0:REMP, :, RSLOT, :], axis=AX, op=OP.add)
                    for s in range(SLOTS):
                        unit(H - 2, s, mh[:, 0, s, :], rs[:, 0, s:s + 1],
                             agg, accs)
                    rem_unit(H - 2, mh, rs[0:REMP, 0, RSLOT:RSLOT + 1], aggr)

                    # head H-1: remainder first, then 3 slot-pair pieces
                    mh = mh_pool.tile([P, 1, SLOTS + 1, N], f32, tag="mh")
                    rs = small_pool.tile([P, 1, SLOTS + 1], f32, tag="rs",
                                         bufs=4)
                    srcr = x[b, H - 1, P * SLOTS:N, :]
                    nc.scalar.dma_start(out=mh[0:REMP, 0, RSLOT, :], in_=srcr)
                    nc.vector.tensor_reduce(
                        out=rs[0:REMP, 0, RSLOT:RSLOT + 1],
                        in_=mh[0:REMP, 0, RSLOT, :], axis=AX, op=OP.add)
                    rem_unit(H - 1, mh, rs[0:REMP, 0, RSLOT:RSLOT + 1], aggr)
                    nc.vector.tensor_reduce(out=nmr, in_=aggr, axis=AX,
                                            op=OP.max, negate=True)
                    nc.scalar.activation(out=aggr, in_=aggr, func=AF.Exp,
                                         bias=nmr, scale=1.0, accum_out=sumr)
                    src = x[b, H - 1, 0:P * SLOTS, :].rearrange(
                        "(p r) j -> p r j", p=P)
                    for c in range(3):
                        sl = slice(2 * c, 2 * c + 2)
                        nc.sync.dma_start(out=mh[:, 0, sl, :], in_=src[:, sl, :])
                        scrp = scr_pool.tile([P, N], f32, tag="scr")
                        for s in (2 * c, 2 * c + 1):
                            nc.scalar.activation(
                                out=scrp, in_=mh[:, 0, s, :], func=AF.Copy,
                                bias=0.0, scale=1.0,
                                accum_out=rs[:, 0, s:s + 1])
                        for s in (2 * c, 2 * c + 1):
                            unit(H - 1, s, mh[:, 0, s, :], rs[:, 0, s:s + 1],
                                 agg, accs, tail=True)
                            finalize_slot(s, agg, accs, nm, sums)
                else:
                    nc.vector.tensor_reduce(out=nmr, in_=aggr, axis=AX,
                                            op=OP.max, negate=True)
                    nc.scalar.activation(out=aggr, in_=aggr, func=AF.Exp,
                                         bias=nmr, scale=1.0, accum_out=sumr)

                # ---- global softmax correction chain for this batch ----
                m1 = small_pool.tile([P, 1], f32, tag="m1", bufs=2)
                nc.vector.tensor_reduce(out=m1, in_=nm, axis=AX, op=OP.min)
                nc.vector.tensor_tensor(out=m1[0:REMP, :], in0=m1[0:REMP, :],
                                        in1=nmr, op=OP.min)
                tps = acc_pool.tile([1, P], f32, tag="acc", name=f"tps_{b}")
                nc.tensor.transpose(tps, m1, ident_sb)
                gmn = small_pool.tile([1, 1], f32, tag="gmn", bufs=2)
                nc.vector.tensor_reduce(out=gmn, in_=tps, axis=AX, op=OP.min)
                bps = acc_pool.tile([P, 1], f32, tag="acc", name=f"bps_{b}")
                nc.tensor.matmul(bps, lhsT=ones_sb[0:1, :], rhs=gmn,
                                 start=True, stop=True)
                negM = small_pool.tile([P, 1], f32, tag="negM", bufs=2)
                nc.vector.tensor_copy(out=negM, in_=bps)
                cfac = small_pool.tile([P, SLOTS], f32, tag="cfac", bufs=2)
                nc.scalar.activation(out=cfac, in_=nm, func=AF.Exp,
                                     bias=negM, scale=-1.0)
                cfr = small_pool.tile([REMP, 1], f32, tag="cfr", bufs=2)
                nc.scalar.activation(out=cfr, in_=nmr, func=AF.Exp,
                                     bias=negM[0:REMP, :], scale=-1.0)
                zc = small_pool.tile([P, SLOTS], f32, tag="zc", bufs=2)
                nc.vector.tensor_tensor(out=zc, in0=sums, in1=cfac, op=OP.mult)
                z1 = small_pool.tile([P, 1], f32, tag="z1", bufs=2)
                nc.vector.tensor_reduce(out=z1, in_=zc, axis=AX, op=OP.add)
                zr = small_pool.tile([REMP, 1], f32, tag="zr", bufs=2)
                nc.vector.tensor_tensor(out=zr, in0=sumr, in1=cfr, op=OP.mult)
                nc.vector.tensor_tensor(out=z1[0:REMP, :], in0=z1[0:REMP, :],
                                        in1=zr, op=OP.add)
                sps = acc_pool.tile([P, 1], f32, tag="acc", name=f"sps_{b}")
                nc.tensor.matmul(sps, lhsT=ones_sb, rhs=z1, start=True,
                                 stop=True)
                rinv = small_pool.tile([P, 1], f32, tag="rinv", bufs=2)
                nc.vector.reciprocal(out=rinv, in_=sps)
                f = small_pool.tile([P, SLOTS], f32, tag="f", bufs=2)
                nc.vector.tensor_scalar_mul(out=f, in0=cfac, scalar1=rinv)
                fr = small_pool.tile([REMP, 1], f32, tag="fr", bufs=2)
                nc.vector.tensor_scalar_mul(out=fr, in0=cfr,
                                            scalar1=rinv[0:REMP, :])

                # ---- final scale (+bf16 cast) and store ----
                dst = y[b, 0:P * SLOTS, :].rearrange("(p r) j -> p r j", p=P)
                for g0 in (0, 3):
                    ot = out_pool.tile([P, 3, N], bf16, tag="out")
                    for k in range(3):
                        s = g0 + k
                        nc.scalar.activation(out=ot[:, k, :],
                                             in_=agg[:, s, :],
                                             func=AF.Copy, bias=0.0,
                                             scale=f[:, s:s + 1])
                    nc.scalar.dma_start(out=dst[:, g0:g0 + 3, :], in_=ot)
                otr = out_pool.tile([REMP, N], bf16, tag="outr", bufs=2)
                nc.scalar.activation(out=otr, in_=aggr, func=AF.Copy,
                                     bias=0.0, scale=fr)
                nc.scalar.dma_start(out=y[b, P * SLOTS:N, :], in_=otr)

    nc.finalize()
    return nc


def kernel(mha_masks) -> np.ndarray:
    global LAST_RESULT
    from concourse.bass_utils import run_bass_kernel_spmd

    xfull = np.ascontiguousarray(np.asarray(mha_masks, dtype=np.float32))
    assert xfull.shape == (B, H, N, N), xfull.shape

    nc = build_program()
    ident = np.eye(P, dtype=np.float32)
    in_maps = [
        {"x": xfull[i * BPC: (i + 1) * BPC], "ident": ident}
        for i in range(NCORES)
    ]
    import os

    kw = {}
    if os.environ.get("KERNEL_TRACE_DIR"):
        kw = dict(trace=True, tmpdir=os.environ["KERNEL_TRACE_DIR"])
    res = run_bass_kernel_spmd(nc, in_maps, core_ids=list(range(NCORES)), **kw)
    LAST_RESULT = res
    out = np.concatenate(
        [np.asarray(r["y"]).astype(np.float32) for r in res.results], axis=0
    )
    return out
